# revision 4
# baseline (speedup 1.0000x reference)
"""ChaRNN LSTM (teacher forcing) Trainium2 Bass kernel.

Structure (data-parallel over batch, 64 rows/core on 8 cores):
  - Teacher forcing means the LSTM input at step t is [f_pool[:,t,:], gt[:,t-1,:]]
    which is fully known ahead of time, so the input projection X @ W_x for all
    32 steps is a big batched matmul (phase A).  Only h @ W_h is sequential
    (phase B).  The argmax/one-hot head is deferred and batched (phase C).
  - All matmuls run in native fp32 (PE 4 cyc/row) - bf16/fp16 single-pass flips
    argmaxes (one-hot output is graded, top-2 logit gaps go down to 6.6e-6).
  - Gate activations use tanh only (4 ULP) via sigmoid(x) = 0.5*tanh(x/2)+0.5;
    the 0.5 pre-scale of the i/f/o gate columns is folded into the weights on
    the host, so one ACT pass computes tanh over the whole 2048-wide gate row.
  - Phase A is emitted interleaved with the recurrence so the tensor engine
    fills the pointwise-tail gaps of each step with input-projection matmuls.
  - Recurrence matmuls pack pairs of K-chunks into PE column halves
    (tile_position (0,0)/(0,64)) since batch=64 only fills half the array;
    measured 1.84x over the unpacked form.
"""

import os
import numpy as np

B, T, DEPTH = 512, 32, 512
RNN, NCC = 512, 128
DIN = DEPTH + NCC            # 640
G4 = 4 * RNN                 # 2048
NCORES = 8
BS = B // NCORES             # 64 batch rows per core
ROWS = T * BS                # 2048 (t-major: r = t*BS + b)

_PROGRAM = None


def _build_program():
    import concourse.bass as bass
    import concourse.tile as tile
    from concourse import bacc, mybir
    from concourse.masks import make_identity

    f32 = mybir.dt.float32
    nc = bacc.Bacc(None)

    xt_d = nc.dram_tensor("xt", [DIN, ROWS], f32, kind="ExternalInput")
    wx_d = nc.dram_tensor("wx", [DIN, G4], f32, kind="ExternalInput")
    wh_d = nc.dram_tensor("wh", [RNN, G4], f32, kind="ExternalInput")
    biasb_d = nc.dram_tensor("biasb", [128, G4], f32, kind="ExternalInput")
    smw_d = nc.dram_tensor("smw", [RNN, NCC], f32, kind="ExternalInput")
    smbb_d = nc.dram_tensor("smbb", [128, NCC], f32, kind="ExternalInput")
    revidx_d = nc.dram_tensor("revidx", [128, NCC], f32, kind="ExternalInput")

    onehot_d = nc.dram_tensor("onehot", [ROWS, NCC], f32, kind="ExternalOutput")
    h_d = nc.dram_tensor("h_out", [BS, RNN], f32, kind="ExternalOutput")
    c_d = nc.dram_tensor("c_out", [BS, RNN], f32, kind="ExternalOutput")

    xt_r = xt_d.rearrange("(kc p) r -> p kc r", p=128)      # [128, 5, ROWS]
    wx_r = wx_d.rearrange("(kc p) g -> p kc g", p=128)      # [128, 5, G4]
    wh_r = wh_d.rearrange("(kc p) g -> p kc g", p=128)      # [128, 4, G4]
    smw_r = smw_d.rearrange("(kc p) n -> p kc n", p=128)    # [128, 4, NCC]

    AluOp = mybir.AluOpType
    Act = mybir.ActivationFunctionType

    with tile.TileContext(nc) as tc:
        with (
            tc.tile_pool(name="persist", bufs=1) as pp,
            tc.tile_pool(name="weights", bufs=1) as wp,
            tc.tile_pool(name="astage", bufs=2) as ap_,
            tc.tile_pool(name="axt", bufs=2) as axt,
            tc.tile_pool(name="bwork", bufs=2) as bp,
            tc.tile_pool(name="bwork1", bufs=1) as bp1,
            tc.tile_pool(name="aps", bufs=2, space="PSUM") as aps,
            tc.tile_pool(name="bps", bufs=1, space="PSUM") as bps,
            tc.tile_pool(name="trps", bufs=2, space="PSUM") as trps,
        ):
            ident = pp.tile([128, 128], f32)
            make_identity(nc, ident[:])
            # HT[:, kc, s*64+b] = h_(s-1)[b, kc*128+p]; block 0 is h0 = 0
            HT = pp.tile([128, 4, (T + 1) * BS], f32)
            nc.gpsimd.memset(HT[:, :, 0:BS], 0.0)

            wx_t = wp.tile([128, 5, G4], f32)
            nc.sync.dma_start(wx_t[:], wx_r[:])
            wh_t = wp.tile([128, 4, G4], f32)
            nc.sync.dma_start(wh_t[:], wh_r[:])
            biasb_t = wp.tile([128, G4], f32)
            nc.sync.dma_start(biasb_t[:], biasb_d[:])

            c_cur = bp.tile([BS, RNN], f32, tag="c")
            nc.gpsimd.memset(c_cur[:], 0.0)

            # Z stage ring: one tile holds one rc-block (128 rows x 2048) of
            # the input projection = 2 timesteps worth.
            def emit_phaseA_block(rc):
                xtrc = axt.tile([128, 5, 128], f32, tag="xtrc")
                nc.sync.dma_start(xtrc[:], xt_r[:, :, rc * 128:(rc + 1) * 128])
                stage = ap_.tile([128, G4], f32, tag="stage")
                for n in range(4):
                    ps = aps.tile([128, 512], f32, tag="aps")
                    for kc in range(5):
                        nc.tensor.matmul(
                            ps[:], xtrc[:, kc, :], wx_t[:, kc, bass.ts(n, 512)],
                            start=(kc == 0), stop=(kc == 4),
                        )
                    nc.vector.tensor_tensor(
                        out=stage[:, bass.ts(n, 512)], in0=ps[:],
                        in1=biasb_t[:, bass.ts(n, 512)], op=AluOp.add,
                    )
                return stage

            stages = {}
            h2 = None
            stages[0] = emit_phaseA_block(0)
            stages[1] = emit_phaseA_block(1)
            next_rc = 2

            for t in range(T):
                rc, half = divmod(t, 2)
                zin = stages[rc][bass.ds(half * BS, BS), :]    # [64, 2048]

                zpa = bps.tile([128, 1024], f32, tag="zpa")
                zpb = bps.tile([128, 1024], f32, tag="zpb")
                for pi, (ka, kb) in enumerate(((0, 1), (2, 3))):
                    for n in range(4):
                        ps = zpa if n < 2 else zpb
                        off = (n % 2) * 512
                        hsl = bass.ds(t * BS, BS)
                        nc.tensor.matmul(
                            ps[0:64, bass.ds(off, 512)],
                            HT[:, ka, hsl], wh_t[:, ka, bass.ts(n, 512)],
                            start=(pi == 0), stop=(pi == 1), tile_position=(0, 0),
                        )
                        nc.tensor.matmul(
                            ps[64:128, bass.ds(off, 512)],
                            HT[:, kb, hsl], wh_t[:, kb, bass.ts(n, 512)],
                            start=(pi == 0), stop=(pi == 1), tile_position=(0, 64),
                        )

                za = bp.tile([BS, G4], f32, tag="za")
                nc.vector.tensor_tensor(out=za[:, 0:1024], in0=zpa[0:64, :],
                                        in1=zin[:, 0:1024], op=AluOp.add)
                nc.vector.tensor_tensor(out=za[:, 0:1024], in0=za[:, 0:1024],
                                        in1=zpa[64:128, :], op=AluOp.add)
                nc.vector.tensor_tensor(out=za[:, 1024:2048], in0=zpb[0:64, :],
                                        in1=zin[:, 1024:2048], op=AluOp.add)
                nc.vector.tensor_tensor(out=za[:, 1024:2048], in0=za[:, 1024:2048],
                                        in1=zpb[64:128, :], op=AluOp.add)
                # za = tanh(z') for all gates (i/f/o columns pre-scaled by 0.5)
                nc.scalar.activation(za[:], za[:], Act.Tanh)

                sif = bp1.tile([BS, 1024], f32, tag="sif")
                nc.vector.tensor_scalar(out=sif[:], in0=za[:, 0:1024],
                                        scalar1=0.5, scalar2=0.5,
                                        op0=AluOp.mult, op1=AluOp.add)
                so = bp1.tile([BS, 512], f32, tag="so")
                nc.gpsimd.tensor_scalar(out=so[:], in0=za[:, 1536:2048],
                                        scalar1=0.5, scalar2=0.5,
                                        op0=AluOp.mult, op1=AluOp.add)
                m1 = bp1.tile([BS, 512], f32, tag="m1")
                nc.gpsimd.tensor_tensor(out=m1[:], in0=sif[:, 512:1024],
                                        in1=c_cur[:], op=AluOp.mult)
                m2 = bp1.tile([BS, 512], f32, tag="m2")
                nc.gpsimd.tensor_tensor(out=m2[:], in0=sif[:, 0:512],
                                        in1=za[:, 1024:1536], op=AluOp.mult)
                c_new = bp.tile([BS, RNN], f32, tag="c")
                nc.vector.tensor_tensor(out=c_new[:], in0=m1[:], in1=m2[:],
                                        op=AluOp.add)
                tc2 = bp1.tile([BS, 512], f32, tag="tc2")
                nc.scalar.activation(tc2[:], c_new[:], Act.Tanh)
                h2 = bp1.tile([BS, RNN], f32, tag="h2")
                nc.vector.tensor_tensor(out=h2[:], in0=so[:], in1=tc2[:],
                                        op=AluOp.mult)

                trp = trps.tile([128, 4, 64], f32, tag="trp")
                for kc in range(4):
                    nc.tensor.transpose(trp[:, kc, :], h2[:, bass.ts(kc, 128)],
                                        ident[0:64, 0:64])
                nc.vector.tensor_copy(HT[:, :, bass.ds((t + 1) * BS, BS)], trp[:])
                c_cur = c_new

                if t % 2 == 0 and next_rc < 16:
                    stages[next_rc] = emit_phaseA_block(next_rc)
                    next_rc += 1

            nc.sync.dma_start(h_d[:], h2[:])
            nc.sync.dma_start(c_d[:], c_cur[:])

            # ---- phase C: logits, argmax (first-index), one-hot ----
            smw_t = wp.tile([128, 4, NCC], f32)
            nc.sync.dma_start(smw_t[:], smw_r[:])
            smb_t = wp.tile([128, NCC], f32)
            nc.sync.dma_start(smb_t[:], smbb_d[:])
            revidx_t = wp.tile([128, NCC], f32)
            nc.sync.dma_start(revidx_t[:], revidx_d[:])

            logits = pp.tile([128, 16, NCC], f32)
            for rc in range(16):
                pl = trps.tile([128, NCC], f32, tag="trp")
                for kc in range(4):
                    nc.tensor.matmul(
                        pl[:], HT[:, kc, bass.ds(BS + rc * 128, 128)],
                        smw_t[:, kc, :], start=(kc == 0), stop=(kc == 3),
                    )
                nc.vector.tensor_tensor(out=logits[:, rc, :], in0=pl[:],
                                        in1=smb_t[:], op=AluOp.add)

            rmax1 = pp.tile([128, 16], f32)
            nc.vector.tensor_reduce(rmax1[:], logits[:], axis=mybir.AxisListType.X,
                                    op=AluOp.max)
            for j in range(16):
                nc.vector.tensor_scalar(out=logits[:, j, :], in0=logits[:, j, :],
                                        scalar1=rmax1[:, j:j + 1], scalar2=None,
                                        op0=AluOp.is_equal)
                nc.vector.tensor_tensor(out=logits[:, j, :], in0=logits[:, j, :],
                                        in1=revidx_t[:], op=AluOp.mult)
            rmax2 = pp.tile([128, 16], f32)
            nc.vector.tensor_reduce(rmax2[:], logits[:], axis=mybir.AxisListType.X,
                                    op=AluOp.max)
            for j in range(16):
                nc.vector.tensor_scalar(out=logits[:, j, :], in0=revidx_t[:],
                                        scalar1=rmax2[:, j:j + 1], scalar2=None,
                                        op0=AluOp.is_equal)
            for rc in range(16):
                nc.sync.dma_start(onehot_d[bass.ts(rc, 128), :], logits[:, rc, :])

    nc.finalize()
    return nc


def _get_program():
    global _PROGRAM
    if _PROGRAM is None:
        _PROGRAM = _build_program()
    return _PROGRAM


def _prep_inputs(f_pool, ground_truth, kernel, rec_kernel, bias, softmax_w,
                 softmax_b):
    # fold the tanh half-angle pre-scale of gates i, f, o into the weights
    col_scale = np.ones((G4,), np.float32)
    col_scale[0 * RNN:2 * RNN] = 0.5     # i, f
    col_scale[3 * RNN:4 * RNN] = 0.5     # o
    wx = np.ascontiguousarray(kernel * col_scale[None, :], np.float32)
    wh = np.ascontiguousarray(rec_kernel * col_scale[None, :], np.float32)
    bias_s = (bias * col_scale).astype(np.float32)
    biasb = np.ascontiguousarray(np.tile(bias_s[None, :], (128, 1)))
    smbb = np.ascontiguousarray(np.tile(softmax_b[None, :].astype(np.float32),
                                        (128, 1)))
    revidx = np.ascontiguousarray(
        np.tile((NCC - np.arange(NCC, dtype=np.float32))[None, :], (128, 1)))
    smw = np.ascontiguousarray(softmax_w, np.float32)

    in_maps = []
    for c in range(NCORES):
        fp = f_pool[c * BS:(c + 1) * BS]          # [64, 32, 512]
        gt = ground_truth[c * BS:(c + 1) * BS]    # [64, 32, 128]
        prev = np.zeros_like(gt)
        prev[:, 1:] = gt[:, :-1]
        fpT = np.ascontiguousarray(fp).transpose(2, 1, 0).reshape(DEPTH, ROWS)
        prT = np.ascontiguousarray(prev).transpose(2, 1, 0).reshape(NCC, ROWS)
        xt = np.ascontiguousarray(np.concatenate([fpT, prT], axis=0))
        in_maps.append({
            "xt": xt, "wx": wx, "wh": wh, "biasb": biasb,
            "smw": smw, "smbb": smbb, "revidx": revidx,
        })
    return in_maps


def _install_ntff_shim():
    """Register the axon NTFF profiling hook the image's antenv lacks."""
    import contextlib, ctypes, sys, types
    try:
        import antenv
    except ImportError:
        return
    if getattr(antenv, "axon_hooks", None) is not None:
        return
    state = {}
    mod = types.ModuleType("antenv.axon_hooks")
    mod.set_axon_ntff_profile_hook = lambda h: state.update(h=h)
    mod.get_axon_ntff_profile_hook = lambda: state.get("h")
    sys.modules["antenv.axon_hooks"] = mod
    antenv.axon_hooks = mod
    try:
        lib = ctypes.CDLL("/opt/axon/libaxon_pjrt.so")
    except OSError:
        return
    if not hasattr(lib, "axon_start_nrt_profile"):
        return
    lib.axon_start_nrt_profile.argtypes = [ctypes.POINTER(ctypes.c_int64),
                                           ctypes.c_size_t]
    lib.axon_start_nrt_profile.restype = ctypes.c_int64
    lib.axon_stop_nrt_profile.argtypes = [ctypes.c_char_p]
    lib.axon_stop_nrt_profile.restype = ctypes.c_int64

    @contextlib.contextmanager
    def _hook(output_dir, device_ids):
        import jax
        jax.devices()
        if device_ids:
            ids = (ctypes.c_int64 * len(device_ids))(*device_ids)
            rc = lib.axon_start_nrt_profile(ids, len(device_ids))
        else:
            rc = lib.axon_start_nrt_profile(None, 0)
        if rc != 0:
            raise RuntimeError(f"axon_start_nrt_profile rc={rc}")
        try:
            yield
        finally:
            n = lib.axon_stop_nrt_profile(str(output_dir).encode())
            if n < 0:
                raise RuntimeError(f"axon_stop_nrt_profile rc={n}")

    mod.set_axon_ntff_profile_hook(_hook)


def _run(in_maps, trace=False):
    from concourse.bass_utils import run_bass_kernel_spmd
    if trace:
        _install_ntff_shim()
    nc = _get_program()
    return run_bass_kernel_spmd(nc, in_maps, list(range(NCORES)), trace=trace)


def kernel(f_pool, ground_truth, kernel, rec_kernel, bias, softmax_w,
           softmax_b):
    f_pool = np.asarray(f_pool, np.float32)
    ground_truth = np.asarray(ground_truth, np.float32)
    in_maps = _prep_inputs(f_pool, ground_truth, np.asarray(kernel, np.float32),
                           np.asarray(rec_kernel, np.float32),
                           np.asarray(bias, np.float32),
                           np.asarray(softmax_w, np.float32),
                           np.asarray(softmax_b, np.float32))
    trace = bool(int(os.environ.get("KERNEL_TRACE", "0")))
    res = _run(in_maps, trace=trace)
    if trace and res.exec_time_ns is not None:
        print(f"HW exec time: {res.exec_time_ns} ns")

    seq = np.empty((B, T, NCC), np.float32)
    h = np.empty((B, RNN), np.float32)
    c = np.empty((B, RNN), np.float32)
    for ci in range(NCORES):
        r = res.results[ci]
        seq[ci * BS:(ci + 1) * BS] = (
            r["onehot"].reshape(T, BS, NCC).transpose(1, 0, 2))
        h[ci * BS:(ci + 1) * BS] = r["h_out"]
        c[ci * BS:(ci + 1) * BS] = r["c_out"]
    return (seq, h, c)
